# revision 1
# baseline (speedup 1.0000x reference)
"""Self-contained Trainium2 Bass kernel for nn_Denoiser_77841987273333.

kernel(**inputs) takes the FULL inputs (as produced by setup_inputs) and returns
the FULL [4, 8192, 3] output. Internally: shards batch*half across 8 NeuronCores
(core = 2*b + half; each core handles 4096 query rows of one batch with the full
8192-point candidate set), compiles one SPMD Bass program, runs it on cores 0-7
via concourse.bass_utils.run_bass_kernel_spmd, and reassembles the output.

Algorithm per core (all on-device):
  f = MLP(x) (3->64->128); z = x_i.x_j - 0.5|x_j|^2 (PE matmuls, PSUM->SBUF);
  exact top-16 neighbors per row via DVE max8/max_index/match_replace (self
  excluded with a -1e30 diagonal); GPSIMD indirect-copy gathers of f and x;
  pair-conv (9->64->64) on PE; attention scores via u = Wk^T q folded so only
  per-pair dot products remain; softmax over 16; weighted sum of neighbor coords.
"""
from contextlib import ExitStack

import numpy as np

import concourse.bass as bass
import concourse.mybir as mybir
import concourse.tile as tile
from concourse.bass_utils import run_bass_kernel_spmd
from concourse.masks import make_identity

F32 = mybir.dt.float32
U16 = mybir.dt.uint16
AF = mybir.ActivationFunctionType
LRELU = 0.01
NEG = -3.0e38
NEGBIG = -1.0e30

B, N, NQ, K = 4, 8192, 4096, 16
N_CORES = 8

WNAMES = ["W1T", "W2T", "Wc1Ta", "Wc1Tb", "Wc1Tc", "Wc1Ts", "Wc2T", "Wc3T",
          "Wc3", "WqT0", "WqT1", "Wk0", "Wk1", "b1c", "b2c", "bc1c", "bc2c",
          "bc3c", "bq0c", "bq1c"]
WSHAPES = [[3, 64], [64, 128], [3, 64], [3, 64], [3, 64], [3, 64], [64, 64],
           [64, 128], [128, 64], [128, 256], [128, 256], [128, 256], [128, 256],
           [64, 1], [128, 1], [64, 1], [64, 1], [128, 1], [128, 1], [128, 1]]


def build(nc: bass.Bass, n=N, nq=NQ):
    n_tiles = nq // 128
    GC = 128 * K
    SB = 128
    C = GC + SB

    xTs_d = nc.dram_tensor("xTs", [128, n], F32, kind="ExternalInput")
    lhq_d = nc.dram_tensor("lhq", [4, nq], F32, kind="ExternalInput")
    wd = {m: nc.dram_tensor(m, s, F32, kind="ExternalInput")
          for m, s in zip(WNAMES, WSHAPES)}
    out_d = nc.dram_tensor("out", [3, nq], F32, kind="ExternalOutput")
    idx_dram = nc.dram_tensor("idx_scratch", [128, K], U16, kind="Internal")
    s_dram = nc.dram_tensor("s_scratch", [1, GC], F32, kind="Internal")
    idx_dramW = nc.dram_tensor("idxw_scratch", [32, 128], U16, kind="Internal")
    idx_dramW8 = nc.dram_tensor("idxw8_scratch", [128, 256], U16, kind="Internal")
    w_rep3 = nc.dram_tensor("wrep3_scratch", [3, GC], F32, kind="Internal")
    w_dram = nc.dram_tensor("w_scratch", [128, K], F32, kind="Internal")

    with tile.TileContext(nc) as tc, ExitStack() as ctx:
        const = ctx.enter_context(tc.tile_pool(name="const", bufs=1))
        xTs = const.tile([128, n], F32)
        lhq = const.tile([4, nq], F32)
        NH = n // 2
        gfA = const.tile([128, NH + 64], F32)
        gfB = const.tile([128, NH + 64], F32)
        gxA = const.tile([128, NH + 64], F32)
        gxB = const.tile([128, NH + 64], F32)
        W = {m: const.tile(s, F32, name=m, tag=m) for m, s in zip(WNAMES, WSHAPES)}
        nbig_id = const.tile([128, 128], F32)
        ones128 = const.tile([128, 1], F32)
        ones64 = const.tile([64, 1], F32)

        for xc in range(8):
            xsl = bass.ts(xc, n // 8)
            nc.sync.dma_start(out=xTs[:, xsl], in_=xTs_d[:, xsl])
        for m in WNAMES:
            nc.sync.dma_start(out=W[m][:], in_=wd[m][:])
        nc.sync.dma_start(out=lhq[:], in_=lhq_d[:])
        for gt in (gfA, gfB, gxA, gxB):
            nc.vector.memset(gt[:], 0.0)
        nc.sync.dma_start(out=gxA[:, 0:NH], in_=xTs_d[:, 0:NH])
        nc.sync.dma_start(out=gxB[:, 1:NH + 1], in_=xTs_d[:, NH:n])
        make_identity(nc, nbig_id[:])
        nc.scalar.mul(out=nbig_id[:], in_=nbig_id[:], mul=NEGBIG)
        nc.vector.memset(ones128[:], 1.0)
        nc.vector.memset(ones64[:], 1.0)

        # f = MLP(x) over all n points
        with tc.tile_pool(name="fmlp_ps", bufs=2, space="PSUM") as fps, \
             tc.tile_pool(name="fmlp_sb", bufs=2) as fsb:
            for c in range(n // 512):
                sl = bass.ts(c, 512)
                p1 = fps.tile([64, 512], F32, tag="p1")
                nc.tensor.matmul(p1[:], W["W1T"][:], xTs[0:3, sl], start=True,
                                 stop=True)
                f1 = fsb.tile([64, 512], F32, tag="f1")
                nc.scalar.activation(f1[:], p1[:], AF.Relu, bias=W["b1c"][:])
                p2 = fps.tile([128, 512], F32, tag="p2")
                nc.tensor.matmul(p2[:], W["W2T"][:], f1[:], start=True, stop=True)
                if c < n // 1024:
                    tgt = gfA[:, bass.ds(c * 512, 512)]
                else:
                    tgt = gfB[:, bass.ds(1 + (c - n // 1024) * 512, 512)]
                nc.scalar.activation(tgt, p2[:], AF.Identity, bias=W["b2c"][:])

        zpool = ctx.enter_context(tc.tile_pool(name="z", bufs=1))
        pools = ctx.enter_context(tc.tile_pool(name="work", bufs=1))
        spool = ctx.enter_context(tc.tile_pool(name="small", bufs=2))
        zps = ctx.enter_context(tc.tile_pool(name="zps", bufs=2, space="PSUM"))
        cps = ctx.enter_context(tc.tile_pool(name="cps", bufs=2, space="PSUM"))

        for t in range(n_tiles):
            trows = bass.ts(t, 128)
            # z = x_i.x_j - 0.5*sq_j
            z = zpool.tile([128, n], F32, tag="z")
            for ch in range(n // 1024):
                zp = zps.tile([128, 1024], F32, tag="zp")
                for h in range(2):
                    sl = bass.ds(ch * 1024 + h * 512, 512)
                    nc.tensor.matmul(zp[:, bass.ts(h, 512)], lhq[:, trows],
                                     xTs[0:4, sl], start=True, stop=True)
                nc.scalar.activation(z[:, bass.ts(ch, 1024)], zp[:], AF.Copy)
            nc.vector.tensor_add(z[:, trows], z[:, trows], nbig_id[:])

            # exact top-16 (values then indices)
            m1 = spool.tile([128, 8], F32, tag="m1")
            m2 = spool.tile([128, 8], F32, tag="m2")
            idx = spool.tile([128, K], U16, tag="idx")
            nc.vector.max(out=m1[:], in_=z[:])
            nc.vector.max_index(out=idx[:, 0:8], in_max=m1[:], in_values=z[:])
            nc.vector.match_replace(out=z[:], in_to_replace=m1[:],
                                    in_values=z[:], imm_value=NEG)
            nc.vector.max(out=m2[:], in_=z[:])
            nc.vector.max_index(out=idx[:, 8:16], in_max=m2[:], in_values=z[:])

            # split indices into half-ranges (sentinel cols are zero)
            idxf = spool.tile([128, 2 * K], F32, tag="idxf")
            nc.vector.tensor_copy(idxf[:, 0:K], idx[:])
            nc.vector.tensor_scalar(idxf[:, 0:K], idxf[:, 0:K], float(NH),
                                    scalar2=None, op0=mybir.AluOpType.min)
            nc.vector.tensor_copy(idxf[:, K:2 * K], idx[:])
            nc.vector.tensor_scalar(idxf[:, K:2 * K], idxf[:, K:2 * K],
                                    float(NH - 1), scalar2=float(-(NH - 1)),
                                    op0=mybir.AluOpType.max,
                                    op1=mybir.AluOpType.add)
            idx2 = spool.tile([128, 2 * K], U16, tag="idx2")
            nc.vector.tensor_copy(idx2[:], idxf[:])
            nc.sync.dma_start(out=idx_dramW.rearrange("p s -> s p"), in_=idx2[:])
            nc.sync.dma_start(
                out=idx_dramW8[:, 0:128].rearrange("(g p) s -> g p s", g=8),
                in_=idx_dramW[0:16][None].broadcast_to([8, 16, 128]))
            nc.sync.dma_start(
                out=idx_dramW8[:, 128:256].rearrange("(g p) s -> g p s", g=8),
                in_=idx_dramW[16:32][None].broadcast_to([8, 16, 128]))
            idxw = spool.tile([128, 256], U16, tag="idxw")
            nc.sync.dma_start(out=idxw[:], in_=idx_dramW8[:])

            GCH = 1024
            kf = pools.tile([128, GC], F32, tag="kf")
            ka = pools.tile([128, GC], F32, tag="ka")
            kb = pools.tile([128, GC], F32, tag="kb")
            for gc in range(GC // GCH):
                gsl = bass.ts(gc, GCH)
                ia = bass.ts(gc, GCH // 16)
                ib = bass.ds(128 + gc * (GCH // 16), GCH // 16)
                nc.gpsimd.indirect_copy(kf[:, gsl], gfA[:], idxw[:, ia], True)
                nc.gpsimd.indirect_copy(kb[:, gsl], gfB[:], idxw[:, ib], True)
            nc.vector.tensor_add(kf[:], kf[:], kb[:])
            for gc in range(GC // GCH):
                gsl = bass.ts(gc, GCH)
                ia = bass.ts(gc, GCH // 16)
                ib = bass.ds(128 + gc * (GCH // 16), GCH // 16)
                nc.gpsimd.indirect_copy(ka[:, gsl], gxA[:], idxw[:, ia], True)
                nc.gpsimd.indirect_copy(kb[:, gsl], gxB[:], idxw[:, ib], True)
            nc.vector.tensor_add(ka[0:4, :], ka[0:4, :], kb[0:4, :])

            # pair convs
            kav = ka[0:3, :].rearrange("c (r j) -> c r j", j=K)
            diff = pools.tile([16, C], F32, tag="diff")
            rep = xTs[0:3, trows].to_broadcast([3, 128, K])
            nc.vector.tensor_sub(
                diff[0:3, 0:GC].rearrange("c (r j) -> c r j", j=K), rep, kav)
            h2 = pools.tile([64, C], F32, tag="h2")
            for cc in range(GC // 512):
                sl = bass.ts(cc, 512)
                p1 = cps.tile([128, 512], F32, tag="cp1")
                rep_c = xTs[0:3, bass.ds(t * 128 + cc * 32, 32)].to_broadcast(
                    [3, 32, K])
                nc.tensor.matmul(p1[0:64, :], W["Wc1Ta"][:], rep_c,
                                 start=True, stop=False)
                nc.tensor.matmul(p1[0:64, :], W["Wc1Tb"][:], ka[0:3, sl],
                                 start=False, stop=False)
                nc.tensor.matmul(p1[0:64, :], W["Wc1Tc"][:], diff[0:3, sl],
                                 start=False, stop=True)
                h1 = spool.tile([64, 512], F32, tag="h1")
                nc.scalar.activation(h1[:], p1[0:64, :], AF.Lrelu,
                                     bias=W["bc1c"][:], alpha=LRELU)
                p2 = cps.tile([128, 512], F32, tag="cp2")
                nc.tensor.matmul(p2[0:64, :], W["Wc2T"][:], h1[:], start=True,
                                 stop=True)
                nc.scalar.activation(h2[:, sl], p2[0:64, :], AF.Lrelu,
                                     bias=W["bc2c"][:], alpha=LRELU)
            p1 = cps.tile([128, 512], F32, tag="cp1")
            nc.tensor.matmul(p1[0:64, 0:SB], W["Wc1Ts"][:], xTs[0:3, trows],
                             start=True, stop=True)
            h1 = spool.tile([64, 512], F32, tag="h1")
            nc.scalar.activation(h1[:, 0:SB], p1[0:64, 0:SB], AF.Lrelu,
                                 bias=W["bc1c"][:], alpha=LRELU)
            p2 = cps.tile([128, 512], F32, tag="cp2")
            nc.tensor.matmul(p2[0:64, 0:SB], W["Wc2T"][:], h1[:, 0:SB],
                             start=True, stop=True)
            nc.scalar.activation(h2[:, bass.ds(GC, SB)], p2[0:64, 0:SB],
                                 AF.Lrelu, bias=W["bc2c"][:], alpha=LRELU)

            # q / u / v
            p1 = cps.tile([128, 512], F32, tag="cp1")
            nc.tensor.matmul(p1[:, 0:128], W["Wc3T"][:], h2[:, bass.ds(GC, SB)],
                             start=True, stop=True)
            rs = spool.tile([128, 128], F32, tag="rs")
            nc.scalar.activation(rs[:], p1[:, 0:128], AF.Identity,
                                 bias=W["bc3c"][:])
            q0 = spool.tile([128, 128], F32, tag="q0")
            q1 = spool.tile([128, 128], F32, tag="q1")
            for h, (qt, bqn) in enumerate([(q0, "bq0c"), (q1, "bq1c")]):
                qp = cps.tile([128, 512], F32, tag="cp2")
                nc.tensor.matmul(qp[:, 0:128], W["WqT0"][:, bass.ts(h, 128)],
                                 gfA[:, trows], start=True, stop=False)
                nc.tensor.matmul(qp[:, 0:128], W["WqT1"][:, bass.ts(h, 128)],
                                 rs[:], start=False, stop=True)
                nc.scalar.activation(qt[:], qp[:, 0:128], AF.Identity,
                                     bias=W[bqn][:])
            u0 = spool.tile([128, 128], F32, tag="u0")
            u1 = spool.tile([128, 128], F32, tag="u1")
            for h, ut in enumerate([u0, u1]):
                up = cps.tile([128, 512], F32, tag="cp1")
                nc.tensor.matmul(up[:, 0:128], W["Wk0"][:, bass.ts(h, 128)],
                                 q0[:], start=True, stop=False)
                nc.tensor.matmul(up[:, 0:128], W["Wk1"][:, bass.ts(h, 128)],
                                 q1[:], start=False, stop=True)
                nc.scalar.activation(ut[:], up[:, 0:128], AF.Copy)
            vp = cps.tile([128, 512], F32, tag="cp2")
            nc.tensor.matmul(vp[0:64, 0:128], W["Wc3"][:], u1[:], start=True,
                             stop=True)
            v = spool.tile([64, 128], F32, tag="v")
            nc.scalar.activation(v[:], vp[0:64, 0:128], AF.Copy)

            # scores s = u0.kf + v.h2
            u0b = u0.to_broadcast([128, 128, K])
            kfv = kf[:].rearrange("c (r j) -> c r j", j=K)
            nc.vector.tensor_mul(kfv, kfv, u0b)
            vb = v.to_broadcast([64, 128, K])
            h2v = h2[:, 0:GC].rearrange("c (r j) -> c r j", j=K)
            nc.vector.tensor_mul(h2v, h2v, vb)
            for cc in range(GC // 512):
                sl = bass.ts(cc, 512)
                sp = cps.tile([128, 512], F32, tag="cp1")
                nc.tensor.matmul(sp[0:1, :], ones128[:], kf[:, sl], start=True,
                                 stop=False)
                nc.tensor.matmul(sp[0:1, :], ones64[:], h2[0:64, sl],
                                 start=False, stop=True)
                nc.scalar.activation(diff[0:1, sl], sp[0:1, :], AF.Copy)

            # softmax over K
            nc.sync.dma_start(out=s_dram[:], in_=diff[0:1, 0:GC])
            st = spool.tile([128, 2 * K + 8], F32, tag="st")
            nc.sync.dma_start(out=st[:, 0:K],
                              in_=s_dram.rearrange("o (r j) -> (o r) j", j=K))
            nc.vector.tensor_reduce(st[:, 2 * K:2 * K + 1], st[:, 0:K],
                                    axis=mybir.AxisListType.X,
                                    op=mybir.AluOpType.max)
            nc.vector.tensor_scalar_mul(st[:, 2 * K + 1:2 * K + 2],
                                        st[:, 2 * K:2 * K + 1], -1.0)
            nc.scalar.activation(st[:, K:2 * K], st[:, 0:K], AF.Exp,
                                 bias=st[:, 2 * K + 1:2 * K + 2])
            nc.vector.tensor_reduce(st[:, 2 * K + 2:2 * K + 3], st[:, K:2 * K],
                                    axis=mybir.AxisListType.X,
                                    op=mybir.AluOpType.add)
            nc.vector.reciprocal(st[:, 2 * K + 3:2 * K + 4],
                                 st[:, 2 * K + 2:2 * K + 3])
            nc.vector.tensor_scalar_mul(st[:, K:2 * K], st[:, K:2 * K],
                                        st[:, 2 * K + 3:2 * K + 4])
            nc.sync.dma_start(out=w_dram[:], in_=st[:, K:2 * K])

            # new_x = sum_j w * knn_x
            nc.sync.dma_start(
                out=w_rep3[:],
                in_=w_dram.rearrange("r j -> (r j)")[None, :].broadcast_to(
                    [3, GC]))
            nc.sync.dma_start(out=diff[0:3, 0:GC], in_=w_rep3[:])
            nc.vector.tensor_mul(ka[0:3, :], ka[0:3, :], diff[0:3, 0:GC])
            nx = spool.tile([16, 128], F32, tag="nx")
            nc.vector.tensor_reduce(nx[0:3, :],
                                    ka[0:3, :].rearrange("c (r j) -> c r j", j=K),
                                    axis=mybir.AxisListType.X,
                                    op=mybir.AluOpType.add)
            nc.sync.dma_start(out=out_d[:, trows], in_=nx[0:3, :])
    return nc


def prep_weights(w: dict):
    Wc1T = np.ascontiguousarray(w["Wc1"].T).astype(np.float32)
    f32 = lambda a: np.ascontiguousarray(a).astype(np.float32)
    return {
        "W1T": f32(w["W1"].T), "W2T": f32(w["W2"].T),
        "Wc1Ta": f32(Wc1T[0:3]), "Wc1Tb": f32(Wc1T[3:6]),
        "Wc1Tc": f32(Wc1T[6:9]), "Wc1Ts": f32(Wc1T[0:3] + Wc1T[3:6]),
        "Wc2T": f32(w["Wc2"].T), "Wc3T": f32(w["Wc3"].T), "Wc3": f32(w["Wc3"]),
        "WqT0": f32(w["Wq"].T[0:128]), "WqT1": f32(w["Wq"].T[128:256]),
        "Wk0": f32(w["Wk"][0:128]), "Wk1": f32(w["Wk"][128:256]),
        "b1c": f32(w["b1"][:, None]), "b2c": f32(w["b2"][:, None]),
        "bc1c": f32(w["bc1"][:, None]), "bc2c": f32(w["bc2"][:, None]),
        "bc3c": f32(w["bc3"][:, None]),
        "bq0c": f32(w["bq"][0:128, None]), "bq1c": f32(w["bq"][128:256, None]),
    }


def prep_xts(x_b: np.ndarray, r0: int, n=N, nq=NQ):
    xr = np.roll(np.asarray(x_b, np.float32), -r0, axis=0)
    xTs = np.zeros((128, n), np.float32)
    for c in range(8):
        xTs[16 * c:16 * c + 3] = xr.T
        xTs[16 * c + 3] = -0.5 * (xr * xr).sum(-1)
    lhq = np.ones((4, nq), np.float32)
    lhq[0:3] = xr.T[:, 0:nq]
    return xTs, lhq


# ---------------------------------------------------------------------------
# Sync legalizer: the walrus in this container encodes at most ~2 sync
# commands per instruction; Tile emits up to 12 inline waits. Split excess
# waits into standalone EventSemaphore instructions (same engine, directly
# before the instruction) — semantically identical (engine blocks on each
# wait in order before issuing).
# ---------------------------------------------------------------------------
import json as _json

import concourse.bass2jax as _bass2jax
import concourse.bass_utils as _bass_utils


def _legalize_sync(bir_json):
    d = _json.loads(bir_json)
    for fn in d["functions"]:
        for bb in fn["blocks"]:
            out = []
            for inst in bb["instructions"]:
                si = inst.get("sync_info")
                waits = (si or {}).get("on_wait") or []
                budget = 1  # keep at most one inline wait per instruction
                if len(waits) > budget:
                    split, keep = waits[:-budget], waits[-budget:]
                    for i, w in enumerate(split):
                        out.append({
                            "debug": inst.get("debug", 0),
                            "engine": inst["engine"],
                            "ins": [], "outs": [],
                            "name": f"{inst['name']}-sw{i}",
                            "opcode": "EventSemaphore",
                            "sync_info": {"on_update": [], "on_wait": [w]},
                        })
                    si["on_wait"] = keep
                out.append(inst)
            bb["instructions"] = out
    return _json.dumps(d).encode()


_orig_compile_bir_kernel = _bass_utils.compile_bir_kernel


def _patched_compile_bir_kernel(bir_json, tmpdir, neff_name="file.neff"):
    return _orig_compile_bir_kernel(_legalize_sync(bir_json), tmpdir,
                                    neff_name=neff_name)


if _bass_utils.compile_bir_kernel is not _patched_compile_bir_kernel:
    _bass_utils.compile_bir_kernel = _patched_compile_bir_kernel
    _bass2jax.compile_bir_kernel = _patched_compile_bir_kernel


_CACHE = {}


def _get_nc():
    if "nc" not in _CACHE:
        nc = bass.Bass("TRN2")
        build(nc)
        _CACHE["nc"] = nc
    return _CACHE["nc"]


def kernel(x, global_feat, W1, b1, W2, b2, Wc1, bc1, Wc2, bc2, Wc3, bc3,
           Wq, bq, Wk, bk, _profile=None):
    del global_feat  # unused by the reference forward
    x = np.asarray(x, np.float32)
    w = prep_weights(dict(W1=W1, b1=b1, W2=W2, b2=b2, Wc1=Wc1, bc1=bc1,
                          Wc2=Wc2, bc2=bc2, Wc3=Wc3, bc3=bc3, Wq=Wq, bq=bq,
                          Wk=Wk, bk=bk))
    in_maps = []
    for core in range(N_CORES):
        b, half = core // 2, core % 2
        m = dict(w)
        m["xTs"], m["lhq"] = prep_xts(x[b], half * NQ)
        in_maps.append(m)

    nc = _get_nc()
    kwargs = dict(_profile) if _profile else {}
    res = run_bass_kernel_spmd(nc, in_maps, core_ids=list(range(N_CORES)),
                               **kwargs)
    out = np.zeros((B, N, 3), np.float32)
    for core in range(N_CORES):
        b, half = core // 2, core % 2
        out[b, half * NQ:(half + 1) * NQ] = res.results[core]["out"].T
    if _profile is not None and isinstance(_profile, dict):
        _profile["exec_time_ns"] = res.exec_time_ns
    return out

